# revision 6
# baseline (speedup 1.0000x reference)
"""Multi-head attention (B=2, N=2048, EMB=1024, H=16, hd=64) on 8 TRN2 NeuronCores.

Sharding: tensor-parallel over heads. Each core owns 2 heads: it gets the
W_qkv columns (k|q|v sections) and W_out rows for those heads, computes
QKV projection + attention + its partial output projection, and the host
sums the 8 partials (the "all-reduce") and adds b_out.

Device kernel layout (per core), all matmuls bf16 with fp32 PSUM accumulation:
  - x is pre-transposed on host to xT [EMB, TOK] so the embedding dim lands on
    SBUF partitions (matmul contraction dim).
  - QKV^T is produced in [dims, tokens] layout: lhsT = W chunk, rhs = xT chunk.
    K/Q sections stay transposed ([hd, tok]) for the scores matmul; the V
    section is staged transposed then PE-transposed into [tok, hd(+ones)]
    tiles (V_aug) for the attn@V matmul.
  - scores^T chunks [k_tok 128, q 512] per head via K=64 matmuls; the two
    heads land in PE row groups 0/64 and execute concurrently. exp on
    ScalarE straight out of PSUM (scale=1/8 folded in, no max subtraction
    needed: scores ~ N(0,1)), bf16 expT.
  - attn@V: lhsT = V_aug [k_tok 128, 65] (col 64 = ones -> row 64 of the
    output accumulates the softmax denominator), accumulated over 16 k chunks.
  - normalize: 1/den on DVE (vector.reciprocal, keeps the chain off the
    exp-saturated ScalarE), partition-broadcast by a K=1 ones-matmul,
    multiply off PSUM on VectorE into A_norm [att 128, tok] bf16.
  - out projection: lhsT = A_norm chunk [128, 128], rhs = W_out shard
    [128, 512], PSUM -> SBUF bf16 -> DRAM partial [TOK, EMB].
  - QKV chunks / V transposes / out-projection tiles are interleaved as
    fine-grained fillers inside the attention chunk loops so the PE stream
    never gaps (PE p-state stays at full clock).
"""

import os

import numpy as np
import ml_dtypes

B = 2
N = 2048
EMB = 1024
TOK = B * N  # 4096
HD = 64
H_PER_CORE = 2
DIMS = 3 * H_PER_CORE * HD  # 384 qkv cols per core
ATT_LOCAL = H_PER_CORE * HD  # 128
P = 128
EC = EMB // P  # 8 embedding chunks
TCQ = TOK // 512  # 8 token chunks for the qkv projection
KCH = N // P  # 16 key chunks per batch
QQ = N // 512  # 4 query quarters per batch
SCALE = HD ** -0.5

_CACHE = {}
LAST = {}


def _build_graph():
    from concourse import bacc, mybir
    import concourse.tile as tile

    nc = bacc.Bacc(
        "TRN2", target_bir_lowering=False, debug=False, num_devices=1
    )
    dt = mybir.dt
    xT = nc.dram_tensor("xT", [EMB, TOK], dt.bfloat16, kind="ExternalInput")
    wqkv = nc.dram_tensor("wqkv", [EMB, DIMS], dt.bfloat16, kind="ExternalInput")
    bqkv = nc.dram_tensor("bqkv", [DIMS], dt.float32, kind="ExternalInput")
    wout = nc.dram_tensor("wout", [ATT_LOCAL, EMB], dt.bfloat16, kind="ExternalInput")
    out = nc.dram_tensor("out", [TOK, EMB], dt.bfloat16, kind="ExternalOutput")

    dbg = {}
    if os.environ.get("KERNEL_DEBUG") == "1":
        dbg["kq"] = nc.dram_tensor(
            "dbg_kq", [2, P, TOK], dt.bfloat16, kind="ExternalOutput"
        )
        dbg["vt"] = nc.dram_tensor(
            "dbg_vt", [P, TOK], dt.bfloat16, kind="ExternalOutput"
        )
        dbg["exp"] = nc.dram_tensor(
            "dbg_exp", [P, 1024], dt.bfloat16, kind="ExternalOutput"
        )
        dbg["anorm"] = nc.dram_tensor(
            "dbg_anorm", [P, TOK], dt.bfloat16, kind="ExternalOutput"
        )

    with tile.TileContext(nc) as tc:
        _emit(tc, nc, xT, wqkv, bqkv, wout, out, dbg)
    nc.compile()
    return nc


def _emit(tc, nc, xT, wqkv, bqkv, wout, out, dbg=None):
    dbg = dbg or {}
    from contextlib import ExitStack
    import concourse.bass as bass
    from concourse import mybir
    from concourse.masks import make_identity

    dt = mybir.dt
    f32, bf16 = dt.float32, dt.bfloat16
    Exp = mybir.ActivationFunctionType.Exp

    with ExitStack() as ctx:
        consts = ctx.enter_context(tc.tile_pool(name="consts", bufs=1))
        xt_pool = ctx.enter_context(tc.tile_pool(name="xt", bufs=3))
        persist = ctx.enter_context(tc.tile_pool(name="persist", bufs=1))
        expp = ctx.enter_context(tc.tile_pool(name="expp", bufs=12))
        small = ctx.enter_context(tc.tile_pool(name="small", bufs=6))
        outst = ctx.enter_context(tc.tile_pool(name="outst", bufs=4))
        ps_scores = ctx.enter_context(
            tc.tile_pool(name="ps_scores", bufs=2, space="PSUM")
        )
        ps_att = ctx.enter_context(tc.tile_pool(name="ps_att", bufs=2, space="PSUM"))
        ps_small = ctx.enter_context(
            tc.tile_pool(name="ps_small", bufs=2, space="PSUM")
        )

        # ---- constants / persistent tiles ----
        # warm up the exp table as early as possible (one-time ~1.3us)
        warm = consts.tile([1, 8], f32, tag="warm")
        nc.vector.memset(warm, 1.0)
        nc.scalar.activation(out=warm, in_=warm, func=Exp, scale=1.0)

        w_sb = consts.tile([P, EC, DIMS], bf16, tag="w_sb")
        for e in range(EC):
            nc.sync.dma_start(out=w_sb[:, e, :], in_=wqkv[e * P : (e + 1) * P, :])
        bias_sb = consts.tile([P, 3], f32, tag="bias_sb")
        nc.sync.dma_start(out=bias_sb, in_=bqkv[:].rearrange("(c p) -> p c", p=P))
        ident = consts.tile([P, P], bf16, tag="ident")
        make_identity(nc, ident)
        ones64 = consts.tile([HD + 1, HD], bf16, tag="ones64")
        nc.vector.memset(ones64, 1.0)
        wm_ps = ps_small.tile([P, 512], f32, tag="ps_small", name="wm_ps")
        for _ in range(28):
            nc.tensor.matmul(
                wm_ps[:, 0:128], lhsT=ident, rhs=ident, start=True, stop=True
            )

        k_sb = persist.tile([P, TOK], bf16, tag="k_sb")
        q_sb = persist.tile([P, TOK], bf16, tag="q_sb")
        vt_sb = persist.tile([P, TOK], bf16, tag="vt_sb")
        # padded to 128 columns so the attn@V weight load gets FWL
        vaug = persist.tile([P, B, H_PER_CORE, KCH, P], bf16, tag="vaug")
        anorm = persist.tile([P, TOK], bf16, tag="anorm")
        nc.vector.memset(vaug[:, :, :, :, :], 0.0)
        # ones column of V_aug (the softmax denominator accumulator row)
        nc.vector.memset(vaug[:, :, :, :, HD : HD + 1], 1.0)

        # wout only feeds the out-projection; its DMA is emitted later (in
        # load_wout below) so it doesn't compete with the startup x loads.
        wout_sb = consts.tile([P, EMB], bf16, tag="wout_sb")

        def load_wout():
            nc.sync.dma_start(out=wout_sb, in_=wout[:, :])

        qkv_dst = (k_sb, q_sb, vt_sb)

        def qkv_dma(t):
            # prefetch xT columns for tokens t*512 .. (t+1)*512
            xt = xt_pool.tile([P, EC, 512], bf16, tag="xt", name=f"xt{t}")
            for e in range(EC):
                nc.sync.dma_start(
                    out=xt[:, e, :], in_=xT[e * P : (e + 1) * P, bass.ts(t, 512)]
                )
            return xt

        def qkv_d(t, d, xt):
            # one of k/q/v for tokens t*512 .. (t+1)*512
            ps = ps_small.tile([P, 512], f32, tag="ps_small", name=f"qkv{t}_{d}")
            for e in range(EC):
                nc.tensor.matmul(
                    ps,
                    lhsT=w_sb[:, e, d * P : (d + 1) * P],
                    rhs=xt[:, e, :],
                    start=(e == 0),
                    stop=(e == EC - 1),
                )
            nc.vector.tensor_scalar_add(
                out=qkv_dst[d][:, bass.ts(t, 512)],
                in0=ps,
                scalar1=bias_sb[:, d : d + 1],
            )

        def alloc_ps_a(b, qq):
            return [
                ps_att.tile([P, 512], f32, tag="ps_att", name=f"ps_a{b}_{qq}_{h}")
                for h in range(H_PER_CORE)
            ]

        def vtrans(b, i0, i1):
            # fill vaug[:, b, h, i, 0:64] = V[tok chunk i, head h] for batch b
            # (PE transpose in bf16, evicted by VectorE)
            for i in range(i0, i1):
                base = b * N + i * P
                for h in range(H_PER_CORE):
                    ps = ps_small.tile([P, 512], bf16, tag="ps_small",
                                       name=f"vt{b}_{i}_{h}")
                    nc.tensor.transpose(
                        ps[:, 0:HD],
                        in_=vt_sb[h * HD : (h + 1) * HD, base : base + P],
                        identity=ident[h * HD : (h + 1) * HD, h * HD : (h + 1) * HD],
                    )
                    nc.vector.tensor_copy(
                        out=vaug[:, b, h, i, 0:HD], in_=ps[:, 0:HD]
                    )

        def attention_chunks(b, qq, ps_a, i0, i1, fillers=()):
            # attn@V for chunk i is emitted two chunks late: by the time the
            # in-order PE stream reaches it, exp(i) has long finished, so the
            # PE never sits on a semaphore between scores matmuls. Filler
            # thunks (qkv pieces, vtrans pieces, outproj tiles) are emitted
            # between chunks to keep every engine fed.
            qbase = b * N + qq * 512
            fillers = list(fillers)
            nfill = len(fillers)
            lag = []

            def attnv(i, ex):
                for h in range(H_PER_CORE):
                    nc.tensor.matmul(
                        ps_a[h],
                        lhsT=vaug[:, b, h, i, :],
                        rhs=ex[:, h * 512 : (h + 1) * 512],
                        start=(i == 0),
                        stop=(i == KCH - 1),
                    )

            nsteps = i1 - i0
            for step, i in enumerate(range(i0, i1)):
                kbase = b * N + i * P
                ps_s = ps_scores.tile([P, 1024], f32, tag="ps_s")
                for h in range(H_PER_CORE):
                    nc.tensor.matmul(
                        ps_s[:, h * 512 : (h + 1) * 512],
                        lhsT=k_sb[h * HD : (h + 1) * HD, kbase : kbase + P],
                        rhs=q_sb[h * HD : (h + 1) * HD, qbase : qbase + 512],
                        start=True,
                        stop=True,
                    )
                ex = expp.tile([P, 1024], bf16, tag="expT")
                nc.scalar.activation(out=ex, in_=ps_s, func=Exp, scale=SCALE)
                if "exp" in dbg and (b, qq, i) == (0, 0, 0):
                    nc.sync.dma_start(out=dbg["exp"][:, :], in_=ex)
                lag.append((i, ex))
                if len(lag) > 2:
                    attnv(*lag.pop(0))
                # spread fillers evenly over the chunk loop
                while fillers and len(fillers) > nfill * (nsteps - 1 - step) // nsteps:
                    fillers.pop(0)()
            for item in lag:
                attnv(*item)
            for f in fillers:
                f()

        def attention_finish(b, qq, ps_a):
            # normalize: 1/den on DVE straight from PSUM (bf16 out),
            # partition-broadcast via a K=1 bf16 ones-matmul, multiplies on
            # VectorE into anorm. No ScalarE involvement: the reciprocal
            # would otherwise queue behind ~4us of bulk exp.
            qbase = b * N + qq * 512
            for h in range(H_PER_CORE):
                psa_sb = small.tile([HD, 512], bf16, tag="psa_sb")
                nc.vector.tensor_copy(out=psa_sb, in_=ps_a[h][0:HD, :])
                rcbf = small.tile([HD + 1, 512], bf16, tag="rcbf")
                with nc.allow_low_precision("bf16 softmax denominator"):
                    nc.vector.reciprocal(
                        out=rcbf[HD : HD + 1, :], in_=ps_a[h][HD : HD + 1, :]
                    )
                rrep_ps = ps_small.tile(
                    [HD, 512], f32, tag="ps_small", name=f"rrep{b}_{qq}_{h}"
                )
                nc.tensor.matmul(
                    rrep_ps,
                    lhsT=ones64[HD : HD + 1, 0:HD],
                    rhs=rcbf[HD : HD + 1, :],
                    start=True,
                    stop=True,
                )
                if h == 0:
                    nc.vector.tensor_mul(
                        out=anorm[0:HD, qbase : qbase + 512],
                        in0=psa_sb,
                        in1=rrep_ps,
                    )
                else:
                    # engine lanes cannot shift partitions; go through a
                    # partition-0 temp and DMA into partitions 64..127.
                    tmp = small.tile([HD, 512], bf16, tag="anorm_tmp")
                    nc.vector.tensor_mul(out=tmp, in0=psa_sb, in1=rrep_ps)
                    nc.sync.dma_start(
                        out=anorm[HD : 2 * HD, qbase : qbase + 512], in_=tmp
                    )

        def outproj_tci(b, qq, tci):
            qbase = b * N + qq * 512
            tok0 = qbase + tci * P
            ob = outst.tile([P, EMB], bf16, tag="outst")
            for e2 in range(2):
                ps = ps_small.tile([P, 512], f32, tag="ps_small",
                                   name=f"op{b}_{qq}_{tci}_{e2}")
                nc.tensor.matmul(
                    ps,
                    lhsT=anorm[:, tok0 : tok0 + P],
                    rhs=wout_sb[:, e2 * 512 : (e2 + 1) * 512],
                    start=True,
                    stop=True,
                )
                nc.vector.tensor_copy(
                    out=ob[:, e2 * 512 : (e2 + 1) * 512], in_=ps
                )
            nc.sync.dma_start(out=out[tok0 : tok0 + P, :], in_=ob)

        # ---- program order ----
        # Unit (0,0) starts after only two QKV chunks (its first 8 score
        # chunks need K/V for tokens 0..1023 only); the rest of QKV, the
        # batch-1 V transposes and the previous unit's out-projection tiles
        # ride as fillers inside the attention chunk loops, so the PE and
        # ScalarE streams never gap.
        xts = {}
        xts[0] = qkv_dma(0)
        xts[1] = qkv_dma(1)
        for d in range(3):
            qkv_d(0, d, xts[0])
        for d in range(3):
            qkv_d(1, d, xts[1])
        xts[2] = qkv_dma(2)
        vtrans(0, 0, 8)
        load_wout()

        def qkv_filler(t, d):
            def f():
                if t not in xts:
                    xts[t] = qkv_dma(t)
                if t + 1 <= TCQ - 1 and t + 1 not in xts:
                    xts[t + 1] = qkv_dma(t + 1)
                qkv_d(t, d, xts[t])
            return f

        ps00 = alloc_ps_a(0, 0)
        attention_chunks(0, 0, ps00, 0, 8,
                         fillers=[qkv_filler(2, d) for d in range(3)])
        vtrans(0, 8, 12)
        attention_chunks(0, 0, ps00, 8, 12,
                         fillers=[qkv_filler(3, 0), qkv_filler(3, 1),
                                  qkv_filler(3, 2)])
        vtrans(0, 12, 16)
        attention_chunks(0, 0, ps00, 12, 16,
                         fillers=[qkv_filler(4, 0)])
        attention_finish(0, 0, ps00)

        # per-unit fillers for the steady-state loop: the previous unit's
        # out-projection tiles, remaining qkv chunks, batch-1 vtrans pieces.
        unit_fillers = {
            (0, 1): [qkv_filler(4, 1), qkv_filler(4, 2), qkv_filler(5, 0)],
            (0, 2): [qkv_filler(5, 1), qkv_filler(5, 2), qkv_filler(6, 0),
                     lambda: vtrans(1, 0, 4)],
            (0, 3): [qkv_filler(6, 1), qkv_filler(6, 2), qkv_filler(7, 0),
                     qkv_filler(7, 1),
                     lambda: vtrans(1, 4, 8), lambda: vtrans(1, 8, 12)],
            (1, 0): [qkv_filler(7, 2), lambda: vtrans(1, 12, 16)],
        }

        prev = (0, 0)
        for b in range(B):
            for qq in range(QQ):
                if (b, qq) == (0, 0):
                    continue
                # unit fillers first: they carry forward-dependencies (qkv
                # pieces feeding vtrans feeding attnv) and must be emitted
                # well before their consumers; outproj tiles have no
                # downstream consumers and can trail.
                fillers = list(unit_fillers.get((b, qq), []))
                for tci in range(4):
                    fillers.append(
                        (lambda bb, qq_, t: lambda: outproj_tci(bb, qq_, t))(
                            prev[0], prev[1], tci
                        )
                    )
                ps_a = alloc_ps_a(b, qq)
                attention_chunks(b, qq, ps_a, 0, KCH, fillers=fillers)
                attention_finish(b, qq, ps_a)
                prev = (b, qq)
        # last unit's out-projection runs at the tail
        for tci in range(4):
            outproj_tci(prev[0], prev[1], tci)

        if dbg:
            nc.sync.dma_start(out=dbg["kq"][0], in_=k_sb[:, :])
            nc.sync.dma_start(out=dbg["kq"][1], in_=q_sb[:, :])
            nc.sync.dma_start(out=dbg["vt"][:, :], in_=vt_sb[:, :])
            nc.sync.dma_start(out=dbg["anorm"][:, :], in_=anorm[:, :])


def _get_graph():
    if "nc" not in _CACHE:
        _CACHE["nc"] = _build_graph()
    return _CACHE["nc"]


def kernel(**inputs):
    x = np.asarray(inputs["x"], dtype=np.float32)
    W_qkv = np.asarray(inputs["W_qkv"], dtype=np.float32)
    b_qkv = np.asarray(inputs["b_qkv"], dtype=np.float32)
    W_out = np.asarray(inputs["W_out"], dtype=np.float32)
    b_out = np.asarray(inputs["b_out"], dtype=np.float32)

    nc = _get_graph()

    bf16 = ml_dtypes.bfloat16
    xT = np.ascontiguousarray(x.reshape(TOK, EMB).T).astype(bf16)
    in_maps = []
    for c in range(8):
        cols = np.concatenate(
            [
                np.arange(c * 128, (c + 1) * 128),
                np.arange(1024 + c * 128, 1024 + (c + 1) * 128),
                np.arange(2048 + c * 128, 2048 + (c + 1) * 128),
            ]
        )
        in_maps.append(
            {
                "xT": xT,
                "wqkv": np.ascontiguousarray(W_qkv[:, cols]).astype(bf16),
                "bqkv": np.ascontiguousarray(b_qkv[cols]).astype(np.float32),
                "wout": np.ascontiguousarray(
                    W_out[c * 128 : (c + 1) * 128, :]
                ).astype(bf16),
            }
        )

    from concourse.bass_utils import run_bass_kernel_spmd

    res = run_bass_kernel_spmd(nc, in_maps, core_ids=list(range(8)))
    LAST["results"] = res

    acc = np.zeros((TOK, EMB), np.float32)
    for r in res.results:
        acc += np.asarray(r["out"], dtype=np.float32)
    acc += b_out[None, :]
    return acc.reshape(B, N, EMB).astype(np.float32)


if __name__ == "__main__":
    rng = np.random.default_rng(0)
    inputs = {
        "x": rng.standard_normal((B, N, EMB), dtype=np.float32),
        "W_qkv": rng.standard_normal((EMB, 3072), dtype=np.float32) * EMB**-0.5,
        "b_qkv": np.zeros((3072,), np.float32),
        "W_out": rng.standard_normal((1024, EMB), dtype=np.float32) * 1024**-0.5,
        "b_out": np.zeros((EMB,), np.float32),
    }
    y = kernel(**inputs)
    print("out", y.shape, y.dtype, float(np.abs(y).mean()))


# revision 44
# speedup vs baseline: 1.1171x; 1.1171x over previous
"""Multi-head attention (B=2, N=2048, EMB=1024, H=16, hd=64) on 8 TRN2 NeuronCores.

Sharding: tensor-parallel over heads. Each core owns 2 heads: it gets the
W_qkv columns (k|q|v sections) and W_out rows for those heads, computes
QKV projection + attention + its partial output projection, and the host
sums the 8 partials (the "all-reduce") and adds b_out.

Device kernel layout (per core), all matmuls bf16 with fp32 PSUM accumulation:
  - x is pre-transposed on host to xT [EMB, TOK] so the embedding dim lands on
    SBUF partitions (matmul contraction dim).
  - QKV^T is produced in [dims, tokens] layout: lhsT = W chunk, rhs = xT chunk.
    K/Q sections stay transposed ([hd, tok]) for the scores matmul; the V
    section is staged transposed then PE-transposed into [tok, hd(+ones)]
    tiles (V_aug) for the attn@V matmul.
  - scores^T chunks [k_tok 128, q 512] per head via K=64 matmuls; the two
    heads land in PE row groups 0/64 and execute concurrently. exp on
    ScalarE straight out of PSUM (scale=1/8 folded in, no max subtraction
    needed: scores ~ N(0,1)), bf16 expT.
  - attn@V: lhsT = V_aug [k_tok 128, 65] (col 64 = ones -> row 64 of the
    output accumulates the softmax denominator), accumulated over 16 k chunks.
  - normalize: 1/den on DVE (vector.reciprocal, keeps the chain off the
    exp-saturated ScalarE), partition-broadcast by a K=1 ones-matmul,
    multiply off PSUM on VectorE into A_norm [att 128, tok] bf16.
  - out projection: lhsT = A_norm chunk [128, 128], rhs = W_out shard
    [128, 512], PSUM -> SBUF bf16 -> DRAM partial [TOK, EMB].
  - QKV chunks / V transposes / out-projection tiles are interleaved as
    fine-grained fillers inside the attention chunk loops so the PE stream
    never gaps (PE p-state stays at full clock).
"""

import os

import numpy as np
import ml_dtypes


def _patch_act_tables():
    # Route Exp to natural_log_exp_and_others so the per-unit Ln calls and
    # the bulk Exp calls share one table set (no ACT_TABLE_LOAD thrash).
    # Entries keep their order, so act_func_set_id indices stay valid.
    import concourse.bacc as bacc_mod
    from concourse import mybir

    if getattr(bacc_mod, "_act_tables_patched", False):
        return
    orig = bacc_mod.get_activation_tables

    def patched(arch):
        t = orig(arch)
        E = mybir.ActivationFunctionType.Exp
        if "natural_log_exp_and_others" in t:
            for name, fns in t.items():
                if name != "natural_log_exp_and_others" and E in fns:
                    t[name] = fns - {E}
        return t

    bacc_mod.get_activation_tables = patched
    bacc_mod._act_tables_patched = True

B = 2
N = 2048
EMB = 1024
TOK = B * N  # 4096
HD = 64
H_PER_CORE = 2
DIMS = 3 * H_PER_CORE * HD  # 384 qkv cols per core
ATT_LOCAL = H_PER_CORE * HD  # 128
P = 128
EC = EMB // P  # 8 embedding chunks
TCQ = TOK // 512  # 8 token chunks for the qkv projection
KCH = N // P  # 16 key chunks per batch
QQ = N // 512  # 4 query quarters per batch
SCALE = HD ** -0.5

_CACHE = {}
LAST = {}


def _build_graph():
    from concourse import bacc, mybir
    import concourse.tile as tile

    _patch_act_tables()
    nc = bacc.Bacc(
        "TRN2", target_bir_lowering=False, debug=False, num_devices=1
    )
    dt = mybir.dt
    xT = nc.dram_tensor("xT", [EMB, TOK], dt.bfloat16, kind="ExternalInput")
    wqkv = nc.dram_tensor("wqkv", [EMB, DIMS], dt.bfloat16, kind="ExternalInput")
    bqkv = nc.dram_tensor("bqkv", [DIMS], dt.float32, kind="ExternalInput")
    wout = nc.dram_tensor("wout", [ATT_LOCAL, EMB], dt.bfloat16, kind="ExternalInput")
    out = nc.dram_tensor("out", [TOK, EMB], dt.bfloat16, kind="ExternalOutput")

    dbg = {}
    if os.environ.get("KERNEL_DEBUG") == "1":
        dbg["kq"] = nc.dram_tensor(
            "dbg_kq", [2, P, TOK], dt.bfloat16, kind="ExternalOutput"
        )
        dbg["vt"] = nc.dram_tensor(
            "dbg_vt", [P, TOK], dt.bfloat16, kind="ExternalOutput"
        )
        dbg["exp"] = nc.dram_tensor(
            "dbg_exp", [P, 1024], dt.bfloat16, kind="ExternalOutput"
        )
        dbg["anorm"] = nc.dram_tensor(
            "dbg_anorm", [P, TOK], dt.bfloat16, kind="ExternalOutput"
        )
        dbg["vaug"] = nc.dram_tensor(
            "dbg_vaug", [P, B * KCH * H_PER_CORE * P], dt.bfloat16,
            kind="ExternalOutput",
        )
        dbg["rc"] = nc.dram_tensor(
            "dbg_rc", [P, 1024], dt.bfloat16, kind="ExternalOutput"
        )

    with tile.TileContext(nc) as tc:
        _emit(tc, nc, xT, wqkv, bqkv, wout, out, dbg)
    nc.compile()
    return nc


def _emit(tc, nc, xT, wqkv, bqkv, wout, out, dbg=None):
    dbg = dbg or {}
    from contextlib import ExitStack
    import concourse.bass as bass
    from concourse import mybir
    from concourse.masks import make_identity

    dt = mybir.dt
    f32, bf16 = dt.float32, dt.bfloat16
    Exp = mybir.ActivationFunctionType.Exp

    with ExitStack() as ctx:
        consts = ctx.enter_context(tc.tile_pool(name="consts", bufs=1))
        xt_pool = ctx.enter_context(tc.tile_pool(name="xt", bufs=3))
        persist = ctx.enter_context(tc.tile_pool(name="persist", bufs=1))
        expp = ctx.enter_context(tc.tile_pool(name="expp", bufs=12))
        small = ctx.enter_context(tc.tile_pool(name="small", bufs=6))
        outst = ctx.enter_context(tc.tile_pool(name="outst", bufs=4))
        ps_scores = ctx.enter_context(
            tc.tile_pool(name="ps_scores", bufs=2, space="PSUM")
        )
        ps_att = ctx.enter_context(tc.tile_pool(name="ps_att", bufs=2, space="PSUM"))
        ps_small = ctx.enter_context(
            tc.tile_pool(name="ps_small", bufs=2, space="PSUM")
        )

        # ---- constants / persistent tiles ----
        # warm up the ln+exp table set as early as possible (one-time ~1.3us)
        Ln = mybir.ActivationFunctionType.Ln
        warm = consts.tile([1, 8], f32, tag="warm")
        nc.vector.memset(warm, 1.0)
        nc.scalar.activation(out=warm, in_=warm, func=Ln, scale=1.0)
        nc.scalar.activation(out=warm, in_=warm, func=Exp, scale=1.0)

        # single batched dma_start per tensor: SP descriptor generation is
        # ~600ns per dma_start and serializes, so 8 separate e-chunk loads
        # would stall the pipeline start by several us.
        w_sb = consts.tile([P, EC, DIMS], bf16, tag="w_sb")
        nc.sync.dma_start(
            out=w_sb[:, :, :],
            in_=wqkv[:, :].rearrange("(e p) d -> p e d", p=P),
        )
        bias_sb = consts.tile([P, 3], f32, tag="bias_sb")
        nc.sync.dma_start(out=bias_sb, in_=bqkv[:].rearrange("(c p) -> p c", p=P))
        ident = consts.tile([P, P], bf16, tag="ident")
        make_identity(nc, ident)
        ones64 = consts.tile([HD + 1, HD], bf16, tag="ones64")
        nc.vector.memset(ones64, 1.0)
        wm_ps = ps_small.tile([P, 512], f32, tag="ps_small", name="wm_ps")
        for _ in range(45):
            nc.tensor.matmul(
                wm_ps[:, 0:128], lhsT=ident, rhs=ident, start=True, stop=True
            )

        k_sb = persist.tile([P, TOK], bf16, tag="k_sb")
        q_sb = persist.tile([P, TOK], bf16, tag="q_sb")
        vt_sb = persist.tile([P, TOK], bf16, tag="vt_sb")
        # per-head blocks padded to 128 columns so the attn@V weight load
        # gets FWL; layout [tok, b, kchunk, head, dims]
        vaug = persist.tile([P, B, KCH, H_PER_CORE, P], bf16, tag="vaug")
        anorm = persist.tile([P, TOK], bf16, tag="anorm")
        # vaug memsets on DVE: on GpSimd they get scheduled between
        # make_identity's two ops and delay the warmup by ~7us; on DVE the
        # 6.9us zero-fill overlaps the initial weight/x DMA window.
        nc.vector.memset(vaug[:, :, :, :, :], 0.0)
        # ones column of V_aug (the softmax denominator accumulator row)
        nc.vector.memset(vaug[:, :, :, :, HD : HD + 1], 1.0)

        # wout only feeds the out-projection; its DMA is emitted later (in
        # load_wout below) so it doesn't compete with the startup x loads.
        wout_sb = consts.tile([P, EMB], bf16, tag="wout_sb")

        def load_wout():
            nc.sync.dma_start(out=wout_sb, in_=wout[:, :])

        qkv_dst = (k_sb, q_sb, vt_sb)

        def qkv_dma(t):
            # prefetch xT columns for tokens t*512 .. (t+1)*512 (one
            # batched dma_start covering all 8 embedding chunks)
            xt = xt_pool.tile([P, EC, 512], bf16, tag="xt", name=f"xt{t}")
            nc.sync.dma_start(
                out=xt[:, :, :],
                in_=xT[:, bass.ts(t, 512)].rearrange("(e p) c -> p e c", p=P),
            )
            return xt

        def qkv_d(t, d, xt):
            # one of k/q/v for tokens t*512 .. (t+1)*512
            ps = ps_small.tile([P, 512], f32, tag="ps_small", name=f"qkv{t}_{d}")
            for e in range(EC):
                nc.tensor.matmul(
                    ps,
                    lhsT=w_sb[:, e, d * P : (d + 1) * P],
                    rhs=xt[:, e, :],
                    start=(e == 0),
                    stop=(e == EC - 1),
                )
            nc.vector.tensor_scalar_add(
                out=qkv_dst[d][:, bass.ts(t, 512)],
                in0=ps,
                scalar1=bias_sb[:, d : d + 1],
            )

        def alloc_ps_a(b, qq):
            return [
                ps_att.tile([P, 512], f32, tag="ps_att", name=f"ps_a{b}_{qq}_{h}")
                for h in range(H_PER_CORE)
            ]

        def vtrans1(b, i):
            # fill vaug[:, b, i, h, 0:64] = V[tok chunk i, head h]
            # (per-head PE transpose in bf16, evicted by VectorE)
            base = b * N + i * P
            for h in range(H_PER_CORE):
                ps = ps_small.tile([P, 512], bf16, tag="ps_small",
                                   name=f"vt{b}_{i}_{h}")
                nc.tensor.transpose(
                    ps[:, 0:HD],
                    in_=vt_sb[h * HD : (h + 1) * HD, base : base + P],
                    identity=ident[h * HD : (h + 1) * HD, h * HD : (h + 1) * HD],
                )
                nc.vector.tensor_copy(
                    out=vaug[:, b, i, h, 0:HD], in_=ps[:, 0:HD]
                )

        def vtrans(b, i0, i1):
            for i in range(i0, i1):
                vtrans1(b, i)

        LAG = 3

        def attention_chunks(b, qq, ps_a, i0, i1, fillers=(), carries=(),
                             lag_state=None, flush=True, fillers_from=0):
            # attn@V for chunk i is emitted LAG chunks late: by the time the
            # in-order PE stream reaches it, exp(i) has long finished, so the
            # PE never sits on a semaphore between scores matmuls. Carry
            # thunks (the previous unit's finish pieces) are emitted one per
            # step starting immediately; filler thunks (qkv pieces, vtrans
            # pieces, outproj tiles) are spread over the whole loop.
            qbase = b * N + qq * 512
            fillers = list(fillers)
            carries = list(carries)
            nfill = len(fillers)
            lag = lag_state if lag_state is not None else []

            def attnv(i, ex):
                for h in range(H_PER_CORE):
                    nc.tensor.matmul(
                        ps_a[h],
                        lhsT=vaug[:, b, i, h, :],
                        rhs=ex[:, h * 512 : (h + 1) * 512],
                        start=(i == 0),
                        stop=(i == KCH - 1),
                    )

            nsteps = i1 - i0
            for step, i in enumerate(range(i0, i1)):
                kbase = b * N + i * P
                ps_s = ps_scores.tile([P, 1024], f32, tag="ps_s")
                for h in range(H_PER_CORE):
                    nc.tensor.matmul(
                        ps_s[:, h * 512 : (h + 1) * 512],
                        lhsT=k_sb[h * HD : (h + 1) * HD, kbase : kbase + P],
                        rhs=q_sb[h * HD : (h + 1) * HD, qbase : qbase + 512],
                        start=True,
                        stop=True,
                    )
                ex = expp.tile([P, 1024], bf16, tag="expT")
                nc.scalar.activation(out=ex, in_=ps_s, func=Exp, scale=SCALE)
                if "exp" in dbg and (b, qq, i) == (0, 0, 0):
                    nc.sync.dma_start(out=dbg["exp"][:, :], in_=ex)
                if carries:
                    carries.pop(0)()
                lag.append((i, ex))
                if len(lag) > LAG:
                    attnv(*lag.pop(0))
                # spread fillers evenly over steps fillers_from..nsteps
                fstep = step - fillers_from
                eff = max(1, nsteps - fillers_from)
                if fstep >= 0:
                    while fillers and len(fillers) > nfill * (eff - 1 - fstep) // eff:
                        fillers.pop(0)()
            if flush:
                while lag:
                    attnv(*lag.pop(0))
            for c in carries:
                c()
            for f in fillers:
                f()

        def finish_a(b, qq, ps_a, h, box):
            # part A: release the PSUM accumulator fast — copy A (bf16) and
            # the denominator row (fp32) to SBUF; both cheap DVE ops.
            psa_sb = small.tile([HD, 512], bf16, tag="psa_sb")
            nc.vector.tensor_copy(out=psa_sb, in_=ps_a[h][0:HD, :])
            den_u = box["den"]
            nc.vector.tensor_copy(
                out=den_u[HD : HD + 1, h * 512 : (h + 1) * 512],
                in_=ps_a[h][HD : HD + 1, :],
            )
            box[h] = psa_sb

        def finish_ln(b, qq, box):
            # 1/den = exp(-ln(den)) on ScalarE, both heads in one pair of
            # [1,1024] activations (same table set as the bulk exp); runs
            # from SBUF so it does not hold the PSUM accumulators.
            Ln = mybir.ActivationFunctionType.Ln
            den_u = box["den"]
            tln = small.tile([HD + 1, 1024], f32, tag="tln")
            nc.scalar.activation(
                out=tln[HD : HD + 1, :], in_=den_u[HD : HD + 1, :], func=Ln
            )
            rcbf = small.tile([HD + 1, 1024], bf16, tag="rcbf")
            nc.scalar.activation(
                out=rcbf[HD : HD + 1, :], in_=tln[HD : HD + 1, :],
                func=Exp, scale=-1.0,
            )
            if "rc" in dbg and (b, qq) == (0, 0):
                nc.sync.dma_start(out=dbg["rc"][HD : HD + 1, :],
                                  in_=rcbf[HD : HD + 1, :])
            box["rc"] = rcbf

        def finish_b(b, qq, h, box):
            # part B: partition-broadcast 1/den via a K=1 bf16 ones-matmul,
            # then one VectorE multiply into anorm.
            psa_sb, rcbf = box[h], box["rc"]
            qbase = b * N + qq * 512
            rrep_ps = ps_small.tile(
                [HD, 512], f32, tag="ps_small", name=f"rrep{b}_{qq}_{h}"
            )
            nc.tensor.matmul(
                rrep_ps,
                lhsT=ones64[HD : HD + 1, 0:HD],
                rhs=rcbf[HD : HD + 1, h * 512 : (h + 1) * 512],
                start=True,
                stop=True,
            )
            if h == 0:
                nc.vector.tensor_mul(
                    out=anorm[0:HD, qbase : qbase + 512],
                    in0=psa_sb,
                    in1=rrep_ps,
                )
            else:
                # engine lanes cannot shift partitions; go through a
                # partition-0 temp and DMA into partitions 64..127.
                tmp = small.tile([HD, 512], bf16, tag="anorm_tmp")
                nc.vector.tensor_mul(out=tmp, in0=psa_sb, in1=rrep_ps)
                nc.sync.dma_start(
                    out=anorm[HD : 2 * HD, qbase : qbase + 512], in_=tmp
                )

        def finish_thunks(b, qq, ps_a):
            # part-As first: they are what releases the PSUM accumulators
            # for the next unit's attn@V.
            box = {"den": None}

            def a(h):
                def f():
                    if box["den"] is None:
                        box["den"] = small.tile(
                            [HD + 1, 1024], f32, tag="den_u",
                            name=f"den{b}_{qq}",
                        )
                    finish_a(b, qq, ps_a, h, box)
                return f

            return [
                a(0),
                a(1),
                lambda: finish_ln(b, qq, box),
                lambda: finish_b(b, qq, 0, box),
                lambda: finish_b(b, qq, 1, box),
            ]

        def attention_finish(b, qq, ps_a):
            for t in finish_thunks(b, qq, ps_a):
                t()

        def outproj_tci(b, qq, tci):
            qbase = b * N + qq * 512
            tok0 = qbase + tci * P
            ob = outst.tile([P, EMB], bf16, tag="outst")
            for e2 in range(2):
                ps = ps_small.tile([P, 512], f32, tag="ps_small",
                                   name=f"op{b}_{qq}_{tci}_{e2}")
                nc.tensor.matmul(
                    ps,
                    lhsT=anorm[:, tok0 : tok0 + P],
                    rhs=wout_sb[:, e2 * 512 : (e2 + 1) * 512],
                    start=True,
                    stop=True,
                )
                nc.vector.tensor_copy(
                    out=ob[:, e2 * 512 : (e2 + 1) * 512], in_=ps
                )
            nc.sync.dma_start(out=out[tok0 : tok0 + P, :], in_=ob)

        # ---- program order ----
        # Unit (0,0) starts as soon as the k/q projections for its first
        # score chunk exist; the v projection, V transposes, remaining QKV
        # chunks, the previous unit's finish pieces (carries) and
        # out-projection tiles all ride inside the attention chunk loops so
        # the PE and ScalarE streams never gap.
        xts = {}
        xts[0] = qkv_dma(0)
        xts[1] = qkv_dma(1)
        xts[2] = qkv_dma(2)
        qkv_d(0, 0, xts[0])
        qkv_d(0, 1, xts[0])
        load_wout()

        def qkv_filler(t, d):
            def f():
                if t not in xts:
                    xts[t] = qkv_dma(t)
                    if t + 1 <= TCQ - 1 and t + 1 not in xts:
                        xts[t + 1] = qkv_dma(t + 1)
                qkv_d(t, d, xts[t])
            return f

        def vt1(b, i):
            return lambda: vtrans1(b, i)

        # fast ramp of unit (0,0): interleave the remaining projections and
        # V transposes at 2-chunk granularity so the first exp issues as
        # early as possible and attn@V inputs always precede their use.
        ps00 = alloc_ps_a(0, 0)
        ramp = [
            (0, 2, [qkv_filler(0, 2), vt1(0, 0), vt1(0, 1)]),
            (2, 4, [vt1(0, 2), vt1(0, 3), qkv_filler(1, 0)]),
            (4, 6, [qkv_filler(1, 1), qkv_filler(1, 2), vt1(0, 4), vt1(0, 5)]),
            (6, 8, [qkv_filler(2, 0), vt1(0, 6), vt1(0, 7)]),
            (8, 10, [qkv_filler(2, 1), qkv_filler(2, 2), vt1(0, 8)]),
            (10, 12, [vt1(0, 9), vt1(0, 10), qkv_filler(3, 0)]),
            (12, 14, [qkv_filler(3, 1), qkv_filler(3, 2), vt1(0, 11),
                      vt1(0, 12)]),
            (14, 16, [vt1(0, 13), vt1(0, 14), vt1(0, 15)]),
        ]
        lag00 = []
        for bi, (i0, i1, fl) in enumerate(ramp):
            attention_chunks(0, 0, ps00, i0, i1, fillers=fl,
                             lag_state=lag00, flush=(bi == len(ramp) - 1))

        # per-unit fillers for the steady-state loop: remaining qkv chunks,
        # batch-1 vtrans pieces, then the previous unit's out-projection.
        unit_fillers = {
            (0, 1): [qkv_filler(4, 0), qkv_filler(4, 1), qkv_filler(4, 2),
                     qkv_filler(5, 0)],
            (0, 2): [qkv_filler(5, 1), qkv_filler(5, 2), qkv_filler(6, 0)]
                    + [vt1(1, i) for i in range(0, 4)],
            (0, 3): [qkv_filler(6, 1), qkv_filler(6, 2), qkv_filler(7, 0),
                     qkv_filler(7, 1)]
                    + [vt1(1, i) for i in range(4, 12)],
            (1, 0): [qkv_filler(7, 2)] + [vt1(1, i) for i in range(12, 16)],
        }

        prev = (0, 0)
        prev_ps = ps00
        for b in range(B):
            for qq in range(QQ):
                if (b, qq) == (0, 0):
                    continue
                fillers = list(unit_fillers.get((b, qq), []))
                for tci in range(4):
                    fillers.append(
                        (lambda bb, qq_, t: lambda: outproj_tci(bb, qq_, t))(
                            prev[0], prev[1], tci
                        )
                    )
                ps_a = alloc_ps_a(b, qq)
                attention_chunks(
                    b, qq, ps_a, 0, KCH, fillers=fillers,
                    carries=finish_thunks(prev[0], prev[1], prev_ps),
                    fillers_from=5,
                )
                prev = (b, qq)
                prev_ps = ps_a
        # last unit finishes inline; its out-projection runs at the tail
        attention_finish(prev[0], prev[1], prev_ps)
        for tci in range(4):
            outproj_tci(prev[0], prev[1], tci)

        if dbg:
            nc.sync.dma_start(out=dbg["kq"][0], in_=k_sb[:, :])
            nc.sync.dma_start(out=dbg["kq"][1], in_=q_sb[:, :])
            nc.sync.dma_start(out=dbg["vt"][:, :], in_=vt_sb[:, :])
            nc.sync.dma_start(out=dbg["anorm"][:, :], in_=anorm[:, :])
            nc.sync.dma_start(
                out=dbg["vaug"][:, :],
                in_=vaug.rearrange("p b c h d -> p (b c h d)"),
            )


def _get_graph():
    if "nc" not in _CACHE:
        _CACHE["nc"] = _build_graph()
    return _CACHE["nc"]


def kernel(**inputs):
    x = np.asarray(inputs["x"], dtype=np.float32)
    W_qkv = np.asarray(inputs["W_qkv"], dtype=np.float32)
    b_qkv = np.asarray(inputs["b_qkv"], dtype=np.float32)
    W_out = np.asarray(inputs["W_out"], dtype=np.float32)
    b_out = np.asarray(inputs["b_out"], dtype=np.float32)

    nc = _get_graph()

    bf16 = ml_dtypes.bfloat16
    xT = np.ascontiguousarray(x.reshape(TOK, EMB).T).astype(bf16)
    in_maps = []
    for c in range(8):
        cols = np.concatenate(
            [
                np.arange(c * 128, (c + 1) * 128),
                np.arange(1024 + c * 128, 1024 + (c + 1) * 128),
                np.arange(2048 + c * 128, 2048 + (c + 1) * 128),
            ]
        )
        in_maps.append(
            {
                "xT": xT,
                "wqkv": np.ascontiguousarray(W_qkv[:, cols]).astype(bf16),
                "bqkv": np.ascontiguousarray(b_qkv[cols]).astype(np.float32),
                "wout": np.ascontiguousarray(
                    W_out[c * 128 : (c + 1) * 128, :]
                ).astype(bf16),
            }
        )

    from concourse.bass_utils import run_bass_kernel_spmd

    res = run_bass_kernel_spmd(nc, in_maps, core_ids=list(range(8)))
    LAST["results"] = res

    acc = np.zeros((TOK, EMB), np.float32)
    for r in res.results:
        acc += np.asarray(r["out"], dtype=np.float32)
    acc += b_out[None, :]
    return acc.reshape(B, N, EMB).astype(np.float32)


if __name__ == "__main__":
    rng = np.random.default_rng(0)
    inputs = {
        "x": rng.standard_normal((B, N, EMB), dtype=np.float32),
        "W_qkv": rng.standard_normal((EMB, 3072), dtype=np.float32) * EMB**-0.5,
        "b_qkv": np.zeros((3072,), np.float32),
        "W_out": rng.standard_normal((1024, EMB), dtype=np.float32) * 1024**-0.5,
        "b_out": np.zeros((EMB,), np.float32),
    }
    y = kernel(**inputs)
    print("out", y.shape, y.dtype, float(np.abs(y).mean()))
